# revision 37
# baseline (speedup 1.0000x reference)
"""CWT (continuous wavelet transform, pywt 'morl', 5 scales) as a Bass/Tile
kernel for 8 Trainium2 NeuronCores.

Math: for each scale s with integrated-wavelet filter k (length L), the
reference computes  trim(diff(full_corr(x, k))) * (-sqrt(s)) along T.  That
whole pipeline is a single correlation with the fixed kernel
    G[j] = sqrt(s) * (k[j] - k[j-1]),  j = 0..L  (k[-1] = k[L] = 0)
applied with offset  off = floor((L-2)/2) - (L-1):
    y[t] = sum_j x[t + off + j] * G[j]   (x zero-padded outside [0,T))
i.e. y = A_s @ x with the Toeplitz band matrix A_s[t, u] = G[u - t - off].

Kernel strategy v2 (SPMD over 8 cores): pure B-sharding.  Core c owns the
128 batch*channel columns [128c, 128c+128); every core computes all 2048
t_out rows for its columns.  All t_out-block indices are then
core-independent, so a single static instruction stream works with NO
per-core shifted data: the banded scales read x chunks directly (chunks
outside [0,16) are the zero-padding and are simply dropped).

All matmul operands are bfloat16 (full PE rate, half the DMA bytes of
fp32r); outputs leave as bf16 too (host upcasts).  ~2.9e-3 rel err vs the
2e-2 budget.  Per (scale, t_out block I):
    psum[b, c0:c1] += X_chunk[q].T @ Wsc[:, w0(s,I,q)+c0 : w0+c1]
accumulated over the chunks q that intersect the band, where Wsc is a
per-scale Toeplitz sliding window identical on every core, and [c0, c1)
is the chunk's exact nonzero band span (PE cost is streamed columns, so
skipping the zero wings of the narrow-band scales cuts the stream from
113k to 92.5k columns; 220 matmuls/core).  start_tensor_calc resets the
whole PSUM bank, so each chain starts with its widest-window chunk
(accumulation commutes) widened to the chain's full column union.

DMA: x (0.5MB) + W (2.3MB, only the band columns actually read) in bf16,
consumption-ordered pieces split across the two HWDGE rings (sync=x+most
outs, scalar=W+late outs); outputs are staged psum->SBUF (vector cast to
bf16) and stored as soon as each chain stops — per-block pieces for
scale 1 (which brackets the schedule), whole-scale stores otherwise.
The dense scales run mid-schedule so their output bursts drain while
later groups compute; the final chain's copy+store is split across both
rings to minimize the tail.
"""
import sys
import os

sys.path.insert(0, "/opt/trn_rl_repo")

import numpy as np

# ----------------------------------------------------------------- constants
WIDTHS = [1, 27, 76, 167, 336]
T = 2048
B = 1024  # 16 batch * 64 channels
N_CORES = 8
BPC = B // N_CORES  # 128 batch*channel columns per core
NQ = T // 128  # 16 t_in chunks
NI = T // 512  # 4 t_out blocks per core (all computed by every core)
NSC = len(WIDTHS)

LAST_EXEC_NS = None  # set when CWT_TRACE=1


def _filters():
    """pywt 'morl' integrated wavelet, resampled per scale (matches reference)."""
    precision = 10
    n = 2**precision
    lb, ub = -8.0, 8.0
    t = np.linspace(lb, ub, n)
    psi = np.exp(-(t**2) / 2.0) * np.cos(5.0 * t)
    step = t[1] - t[0]
    int_psi = np.cumsum(psi) * step
    filts = []
    for scale in WIDTHS:
        j = (np.arange(scale * (ub - lb) + 1) / (scale * step)).astype(np.int64)
        j = j[j < n]
        filts.append(int_psi[j].astype(np.float32))
    return filts


def _g_kernels():
    """Effective correlation kernels G_s (len L+1) and offsets off_s."""
    gs = []
    for s, k in zip(WIDTHS, _filters()):
        k64 = k.astype(np.float64)
        L = len(k64)
        G = (np.sqrt(s) * np.diff(np.concatenate([[0.0], k64, [0.0]]))).astype(
            np.float32
        )
        off = int(np.floor((L - 2) / 2.0)) - (L - 1)
        gs.append((G, off))
    return gs


def _plan():
    """Per-scale Toeplitz window geometry + per-block chunk ranges.

    w0(s, I, q) = C_s + off_s - (128q - 512I) is the first W column of the
    512-wide rhs slice for chunk q of t_out block I.
    """
    plans = []
    for G, off in _g_kernels():
        L1 = len(G)
        qr = []
        vs = []
        for I in range(NI):
            lo = max(0, (512 * I + off) // 128)
            hi = min(NQ - 1, (512 * I + 511 + off + L1 - 1) // 128)
            qr.append((lo, hi))
            vs += [128 * q - 512 * I for q in range(lo, hi + 1)]
        C = max(vs) - off
        W = max(vs) - min(vs) + 512
        plans.append({"off": off, "L1": L1, "qr": qr, "C": C, "W": W, "G": G})
    return plans


def _toeplitz(G, C, W):
    p = np.arange(128)[:, None]
    w = np.arange(W)[None, :]
    idx = p - w + C
    valid = (idx >= 0) & (idx < len(G))
    return np.where(valid, G[np.clip(idx, 0, len(G) - 1)], np.float32(0.0)).astype(
        np.float32
    )


# chain processing groups: (scale, t_out block).  Scale 1's short chains
# bracket the schedule: two start it (smallest W piece -> earliest first
# matmul) and two end it (shortest chains -> smallest output tail).  The
# dense scales run mid-schedule so their 4-chains-end-together output
# bursts drain while later groups compute.
# chain spec: (scale, t_out block, psum col lo, col hi).  The opening
# chain (0,0) is split into two half-width chains so the first W DMA piece
# on the critical path to the first matmul is only 256 cols.
GROUPS = [
    [(0, 0, 0, 256), (0, 0, 256, 512), (0, 1, 0, 512)],
    [(1, 0, 0, 512), (1, 1, 0, 512), (1, 2, 0, 512), (1, 3, 0, 512)],
    [(3, 0, 0, 512), (3, 1, 0, 512), (3, 2, 0, 512), (3, 3, 0, 512)],
    [(4, 0, 0, 512), (4, 1, 0, 512), (4, 2, 0, 512), (4, 3, 0, 512)],
    [(2, 0, 0, 512), (2, 1, 0, 512), (2, 2, 0, 512), (2, 3, 0, 512)],
    [(0, 2, 0, 512), (0, 3, 0, 512)],
]
# psum bank tag per chain, alternating the two 4-bank halves between
# consecutive groups
GROUP_TAGS = [[0, 1, 2], [4, 5, 6, 7], [0, 1, 2, 3], [4, 5, 6, 7], [0, 1, 2, 3], [4, 5]]


def _chain_windows(p, I, cl=0, ch=512):
    """Per-chunk banded column windows for one (scale, block) chain,
    restricted to psum cols [cl, ch).

    Returns [(q, c0, c1, start, stop)]: matmul psum cols [c0, c1), skipping
    the all-zero columns of the Toeplitz band slice (big win for the
    narrow-band scales).
    """
    C, L1, off = p["C"], p["L1"], p["off"]
    nz_lo, nz_hi = C - L1 + 1, C + 128  # nonzero W cols [lo, hi)
    lo, hi = p["qr"][I]
    spans = {}
    for q in range(lo, hi + 1):
        w0 = C + off - (128 * q - 512 * I)
        a = max(cl, nz_lo - w0)
        b = min(ch, nz_hi - w0)
        if b <= a:
            continue
        spans[q] = (a, b)  # exact band span; matmul N needs no alignment
    # start_tensor_calc resets the ENTIRE psum bank on hardware, so each
    # chain gets exactly one start.  Accumulation is commutative: lead with
    # the chunk whose band window is widest (interior chunks cover the full
    # 512, so widening the start to the chain's column union is ~free) and
    # append the remaining chunks in q order.  stop rides the final matmul.
    qs = list(spans)
    qstar = max(qs, key=lambda q: spans[q][1] - spans[q][0])
    cmin = min(a for a, _ in spans.values())
    cmax = max(b for _, b in spans.values())
    rest = [q for q in qs if q != qstar]
    out = [(qstar, cmin, cmax, True, not rest)]
    for i, q in enumerate(rest):
        a, b = spans[q]
        out.append((q, a, b, False, i == len(rest) - 1))
    return out


def _schedule(plans):
    """Emission-ordered list of (chain, q, w0, c0, c1, start, stop,
    chain_done) where chain = (s, I, cl, ch).

    Chains in a group are interleaved round-robin by window index, which
    preserves each chain's start-first order while pacing x-chunk arrival.
    """
    sched = []
    for gi, chains in enumerate(GROUPS):
        wins = {c: _chain_windows(plans[c[0]], c[1], c[2], c[3]) for c in chains}
        if gi <= 1:
            # the opening groups run while x pieces are still arriving;
            # serial chains consume chunks in arrival order (round-robin
            # would front-load all four interior start-chunks)
            order = [(c, i) for c in chains for i in range(len(wins[c]))]
        else:
            maxlen = max(len(v) for v in wins.values())
            order = [
                (c, step)
                for step in range(maxlen)
                for c in chains
                if step < len(wins[c])
            ]
        for c, step in order:
            q, c0, c1, st, sp = wins[c][step]
            s, I = c[0], c[1]
            w0 = plans[s]["C"] + plans[s]["off"] - (128 * q - 512 * I)
            sched.append((c, q, w0, c0, c1, st, sp, step == len(wins[c]) - 1))
    return sched


_CONST_CACHE = None


def _consts():
    global _CONST_CACHE
    if _CONST_CACHE is None:
        import ml_dtypes

        plans = _plan()
        wbuf = np.concatenate(
            [_toeplitz(p["G"], p["C"], p["W"]) for p in plans], axis=1
        ).astype(ml_dtypes.bfloat16)
        _CONST_CACHE = (plans, np.ascontiguousarray(wbuf))
    return _CONST_CACHE


# ----------------------------------------------------------------- program
_NC_CACHE = None


def _w_pieces(plans, sched):
    """Split the concatenated W buffer into DMA pieces, ordered by first use.

    Scale 0's first piece is cut to exactly cover its first two schedule
    steps so the PE can start ~1us sooner; other scales use 2 halves
    (few pieces -> few DMA sems -> short semaphore-teardown epilogue).
    """
    bases = []
    b = 0
    for p in plans:
        bases.append(b)
        b += p["W"]
    pieces = []  # (first_use_idx, lo, hi) in concat cols
    for s, p in enumerate(plans):
        # only the columns the windowed schedule actually reads
        rd = [(w0 + c0, w0 + c1) for c, q, w0, c0, c1, st, sp, cd in sched if c[0] == s]
        rlo = min(a for a, b in rd)
        rhi = max(b for a, b in rd)
        npieces = max(1, round((rhi - rlo) / 1024))
        step = (((rhi - rlo) // npieces) + 127) & ~127
        cuts = list(range(rlo, rhi, step)) + [rhi]
        if s == 0:
            # extra cut so the opening half-chain's first read is its own
            # tiny piece
            f = sched[0]
            assert f[0][0] == 0
            fl, fh = f[2] + f[3], f[2] + f[4]
            cuts = sorted({rlo, fl, fh, rhi} | set(cuts))
        for lo, hi in zip(cuts[:-1], cuts[1:]):
            first = None
            for i, (c, q, w0, c0, c1, st, sp, cd) in enumerate(sched):
                if c[0] == s and w0 + c0 < hi and w0 + c1 > lo:
                    first = i
                    break
            pieces.append((first if first is not None else len(sched), bases[s] + lo, bases[s] + hi))
    pieces.sort()
    return bases, [(lo, hi) for _, lo, hi in pieces]


def _build_program():
    import concourse.bass as bass
    import concourse.bacc as bacc
    import concourse.mybir as mybir
    import concourse.tile as tile

    plans, _ = _consts()
    sched = _schedule(plans)
    wtot = sum(p["W"] for p in plans)
    bases, wpieces = _w_pieces(plans, sched)

    nc = bacc.Bacc(None, target_bir_lowering=False, debug=False)

    x_d = nc.declare_dram_parameter("x", [128, NQ * BPC], mybir.dt.bfloat16, isOutput=False)
    w_d = nc.declare_dram_parameter("w", [128, wtot], mybir.dt.bfloat16, isOutput=False)
    # outputs leave as bf16 (host upcasts): halves the store bytes; adds
    # ~0.1% rms quantization vs the 2e-2 budget
    out_d = nc.declare_dram_parameter(
        "out", [NSC, 128, T], mybir.dt.bfloat16, isOutput=True
    )

    # ring for each whole-scale output store (sync carries x early, scalar
    # carries W early; both are free by the time these flow mid-schedule)
    OUT_ENG = {1: "sync", 3: "sync", 4: "scalar", 2: "scalar"}
    # (0,2) has the most windows in the final group, so round-robin emission
    # makes it the very last chain to finish
    LAST_CHAIN = (0, 2, 0, 512)

    with tile.TileContext(nc) as tc:
        with (
            tc.tile_pool(name="xp", bufs=1) as xp,
            tc.tile_pool(name="wp", bufs=1) as wp,
            tc.tile_pool(name="op", bufs=1) as op,
            tc.tile_pool(name="pp", bufs=1, space=bass.MemorySpace.PSUM) as pp,
        ):
            # x chunks on the sync (SP) HWDGE ring in consumption order
            xsb = xp.tile([128, NQ * BPC], mybir.dt.bfloat16, tag="xsb", name="xsb")
            for g0, g1 in ((0, 3), (3, 7), (7, 11), (11, NQ)):
                nc.sync.dma_start(
                    xsb[:, g0 * BPC : g1 * BPC],
                    x_d[:, g0 * BPC : g1 * BPC],
                )

            # W pieces on the scalar (ACT) ring, first-use order
            wsb = wp.tile([128, wtot], mybir.dt.bfloat16, tag="wsb", name="wsb")
            for lo, hi in wpieces:
                nc.scalar.dma_start(wsb[:, lo:hi], w_d[:, lo:hi])

            # PE clock warmup: the PE sits idle ~2.5us waiting for the first
            # DMA while DVFS has it at low clock (first real matmuls
            # otherwise run 370-700ns instead of 216).  Dummy matmuls on a
            # zeroed scratch tile ramp it; every real chain opens with
            # start=True so the scratch psum bank contents never leak.
            warm = xp.tile([128, 512], mybir.dt.bfloat16, tag="warm", name="warm")
            nc.gpsimd.memset(warm[:], 0.0)
            wps = pp.tile([128, 512], mybir.dt.float32, tag="ps7", name="ps_warm")
            for _ in range(4):
                nc.tensor.matmul(
                    wps[:], warm[:, 0:128], warm[:], start=True, stop=True
                )

            stgs = [
                op.tile([128, T], mybir.dt.bfloat16, tag=f"stg{s}", name=f"stg{s}")
                for s in range(NSC)
            ]

            psums = {}
            for gi, chains in enumerate(GROUPS):
                for ci, c in enumerate(chains):
                    psums[c] = pp.tile(
                        [128, 512],
                        mybir.dt.float32,
                        tag=f"ps{GROUP_TAGS[gi][ci]}",
                        name=f"ps_{c[0]}_{c[1]}_{c[2]}",
                    )

            done = {s: 0 for s in range(NSC)}
            for c, q, w0, c0, c1, start, stop, chain_done in sched:
                s, I, cl, ch = c
                nc.tensor.matmul(
                    psums[c][:, c0:c1],
                    xsb[:, q * BPC : (q + 1) * BPC],
                    wsb[:, bases[s] + w0 + c0 : bases[s] + w0 + c1],
                    start=start,
                    stop=stop,
                )
                if not chain_done:
                    continue
                stg = stgs[s]
                t0c = 512 * I
                done[s] += 1
                if c == LAST_CHAIN:
                    # final chain: copy halves, stores on both rings, to
                    # shorten the tail
                    nc.vector.tensor_copy(
                        stg[:, t0c : t0c + 256], psums[c][:, 0:256]
                    )
                    nc.vector.tensor_copy(
                        stg[:, t0c + 256 : t0c + 512], psums[c][:, 256:512]
                    )
                    nc.sync.dma_start(
                        out_d[s][:, t0c : t0c + 256], stg[:, t0c : t0c + 256]
                    )
                    nc.scalar.dma_start(
                        out_d[s][:, t0c + 256 : t0c + 512],
                        stg[:, t0c + 256 : t0c + 512],
                    )
                    continue
                nc.vector.tensor_copy(
                    stg[:, t0c + cl : t0c + ch], psums[c][:, cl:ch]
                )
                if s == 0:
                    # scale 1 brackets the schedule; store per-piece
                    nc.sync.dma_start(
                        out_d[s][:, t0c + cl : t0c + ch], stg[:, t0c + cl : t0c + ch]
                    )
                elif done[s] == NI:
                    # whole-scale store once the last block is staged
                    eng = nc.sync if OUT_ENG[s] == "sync" else nc.scalar
                    eng.dma_start(out_d[s], stg[:])

    nc.compile()
    return nc


def _program():
    global _NC_CACHE
    if _NC_CACHE is None:
        _NC_CACHE = _build_program()
    return _NC_CACHE


# ----------------------------------------------------------------- entry
def kernel(x: np.ndarray) -> np.ndarray:
    """x: [16, 2048, 64] float32 -> [16, 2048, 64, 5] float32"""
    global LAST_EXEC_NS
    import ml_dtypes
    from concourse.bass_utils import run_bass_kernel_spmd

    x = np.asarray(x)
    n, t, c = x.shape
    assert (t, n * c) == (T, B), (x.shape,)

    X = x.transpose(1, 0, 2).reshape(T, B).astype(np.float32)
    _, wbuf = _consts()
    in_maps = []
    for core in range(N_CORES):
        xc = X[:, core * BPC : (core + 1) * BPC]  # [2048, 128]
        xc = (
            xc.reshape(NQ, 128, BPC)
            .transpose(1, 0, 2)
            .reshape(128, NQ * BPC)
            .astype(ml_dtypes.bfloat16)
        )
        in_maps.append({"x": np.ascontiguousarray(xc), "w": wbuf})

    nc = _program()
    trace = bool(int(os.environ.get("CWT_TRACE", "0")))
    res = run_bass_kernel_spmd(nc, in_maps, list(range(N_CORES)), trace=trace)
    if trace:
        LAST_EXEC_NS = res.exec_time_ns
        globals()["LAST_RESULTS"] = res

    # per-core out: [5, 128, 2048] bf16 (b-local, t) -> Y [5, T, B] fp32
    Y = np.empty((NSC, T, B), np.float32)
    for core in range(N_CORES):
        o = np.asarray(res.results[core]["out"]).astype(np.float32)
        Y[:, :, core * BPC : (core + 1) * BPC] = o.transpose(0, 2, 1)
    return np.ascontiguousarray(
        Y.reshape(NSC, T, n, c).transpose(2, 1, 3, 0).astype(np.float32)
    )


# revision 38
# speedup vs baseline: 1.0148x; 1.0148x over previous
"""CWT (continuous wavelet transform, pywt 'morl', 5 scales) as a Bass/Tile
kernel for 8 Trainium2 NeuronCores.

Math: for each scale s with integrated-wavelet filter k (length L), the
reference computes  trim(diff(full_corr(x, k))) * (-sqrt(s)) along T.  That
whole pipeline is a single correlation with the fixed kernel
    G[j] = sqrt(s) * (k[j] - k[j-1]),  j = 0..L  (k[-1] = k[L] = 0)
applied with offset  off = floor((L-2)/2) - (L-1):
    y[t] = sum_j x[t + off + j] * G[j]   (x zero-padded outside [0,T))
i.e. y = A_s @ x with the Toeplitz band matrix A_s[t, u] = G[u - t - off].

Kernel strategy v2 (SPMD over 8 cores): pure B-sharding.  Core c owns the
128 batch*channel columns [128c, 128c+128); every core computes all 2048
t_out rows for its columns.  All t_out-block indices are then
core-independent, so a single static instruction stream works with NO
per-core shifted data: the banded scales read x chunks directly (chunks
outside [0,16) are the zero-padding and are simply dropped).

All matmul operands are bfloat16 (full PE rate, half the DMA bytes of
fp32r); outputs leave as bf16 too (host upcasts).  ~2.9e-3 rel err vs the
2e-2 budget.  Per (scale, t_out block I):
    psum[b, c0:c1] += X_chunk[q].T @ Wsc[:, w0(s,I,q)+c0 : w0+c1]
accumulated over the chunks q that intersect the band, where Wsc is a
per-scale Toeplitz sliding window identical on every core, and [c0, c1)
is the chunk's exact nonzero band span (PE cost is streamed columns, so
skipping the zero wings of the narrow-band scales cuts the stream from
113k to 92.5k columns; 220 matmuls/core).  start_tensor_calc resets the
whole PSUM bank, so each chain starts with its widest-window chunk
(accumulation commutes) widened to the chain's full column union.

DMA: x (0.5MB) + W (2.3MB, only the band columns actually read) in bf16,
consumption-ordered pieces split across the two HWDGE rings (sync=x+most
outs, scalar=W+late outs); outputs are staged psum->SBUF (vector cast to
bf16) and stored as soon as each chain stops — per-block pieces for
scale 1 (which brackets the schedule), whole-scale stores otherwise.
The dense scales run mid-schedule so their output bursts drain while
later groups compute; the final chain's copy+store is split across both
rings to minimize the tail.
"""
import sys
import os

sys.path.insert(0, "/opt/trn_rl_repo")

import numpy as np

# ----------------------------------------------------------------- constants
WIDTHS = [1, 27, 76, 167, 336]
T = 2048
B = 1024  # 16 batch * 64 channels
N_CORES = 8
BPC = B // N_CORES  # 128 batch*channel columns per core
NQ = T // 128  # 16 t_in chunks
NI = T // 512  # 4 t_out blocks per core (all computed by every core)
NSC = len(WIDTHS)

LAST_EXEC_NS = None  # set when CWT_TRACE=1


def _filters():
    """pywt 'morl' integrated wavelet, resampled per scale (matches reference)."""
    precision = 10
    n = 2**precision
    lb, ub = -8.0, 8.0
    t = np.linspace(lb, ub, n)
    psi = np.exp(-(t**2) / 2.0) * np.cos(5.0 * t)
    step = t[1] - t[0]
    int_psi = np.cumsum(psi) * step
    filts = []
    for scale in WIDTHS:
        j = (np.arange(scale * (ub - lb) + 1) / (scale * step)).astype(np.int64)
        j = j[j < n]
        filts.append(int_psi[j].astype(np.float32))
    return filts


def _g_kernels():
    """Effective correlation kernels G_s (len L+1) and offsets off_s."""
    gs = []
    for s, k in zip(WIDTHS, _filters()):
        k64 = k.astype(np.float64)
        L = len(k64)
        G = (np.sqrt(s) * np.diff(np.concatenate([[0.0], k64, [0.0]]))).astype(
            np.float32
        )
        off = int(np.floor((L - 2) / 2.0)) - (L - 1)
        gs.append((G, off))
    return gs


def _plan():
    """Per-scale Toeplitz window geometry + per-block chunk ranges.

    w0(s, I, q) = C_s + off_s - (128q - 512I) is the first W column of the
    512-wide rhs slice for chunk q of t_out block I.
    """
    plans = []
    for G, off in _g_kernels():
        L1 = len(G)
        qr = []
        vs = []
        for I in range(NI):
            lo = max(0, (512 * I + off) // 128)
            hi = min(NQ - 1, (512 * I + 511 + off + L1 - 1) // 128)
            qr.append((lo, hi))
            vs += [128 * q - 512 * I for q in range(lo, hi + 1)]
        C = max(vs) - off
        W = max(vs) - min(vs) + 512
        plans.append({"off": off, "L1": L1, "qr": qr, "C": C, "W": W, "G": G})
    return plans


def _toeplitz(G, C, W):
    p = np.arange(128)[:, None]
    w = np.arange(W)[None, :]
    idx = p - w + C
    valid = (idx >= 0) & (idx < len(G))
    return np.where(valid, G[np.clip(idx, 0, len(G) - 1)], np.float32(0.0)).astype(
        np.float32
    )


# chain processing groups: (scale, t_out block).  Scale 1's short chains
# bracket the schedule: two start it (smallest W piece -> earliest first
# matmul) and two end it (shortest chains -> smallest output tail).  The
# dense scales run mid-schedule so their 4-chains-end-together output
# bursts drain while later groups compute.
# chain spec: (scale, t_out block, psum col lo, col hi).  The opening
# chain (0,0) is split into two half-width chains so the first W DMA piece
# on the critical path to the first matmul is only 256 cols.
GROUPS = [
    [(0, 0, 0, 256), (0, 0, 256, 512), (0, 1, 0, 512)],
    [(1, 0, 0, 512), (1, 1, 0, 512), (1, 2, 0, 512), (1, 3, 0, 512)],
    [(3, 0, 0, 512), (3, 1, 0, 512), (3, 2, 0, 512), (3, 3, 0, 512)],
    [(4, 0, 0, 512), (4, 1, 0, 512), (4, 2, 0, 512), (4, 3, 0, 512)],
    [(2, 0, 0, 512), (2, 1, 0, 512), (2, 2, 0, 512), (2, 3, 0, 512)],
    [(0, 2, 0, 512), (0, 3, 0, 512)],
]
# psum bank tag per chain, alternating the two 4-bank halves between
# consecutive groups
GROUP_TAGS = [[0, 1, 2], [4, 5, 6, 7], [0, 1, 2, 3], [4, 5, 6, 7], [0, 1, 2, 3], [4, 5]]


def _chain_windows(p, I, cl=0, ch=512):
    """Per-chunk banded column windows for one (scale, block) chain,
    restricted to psum cols [cl, ch).

    Returns [(q, c0, c1, start, stop)]: matmul psum cols [c0, c1), skipping
    the all-zero columns of the Toeplitz band slice (big win for the
    narrow-band scales).
    """
    C, L1, off = p["C"], p["L1"], p["off"]
    nz_lo, nz_hi = C - L1 + 1, C + 128  # nonzero W cols [lo, hi)
    lo, hi = p["qr"][I]
    spans = {}
    for q in range(lo, hi + 1):
        w0 = C + off - (128 * q - 512 * I)
        a = max(cl, nz_lo - w0)
        b = min(ch, nz_hi - w0)
        if b <= a:
            continue
        spans[q] = (a, b)  # exact band span; matmul N needs no alignment
    # start_tensor_calc resets the ENTIRE psum bank on hardware, so each
    # chain gets exactly one start.  Accumulation is commutative: lead with
    # the chunk whose band window is widest (interior chunks cover the full
    # 512, so widening the start to the chain's column union is ~free) and
    # append the remaining chunks in q order.  stop rides the final matmul.
    qs = list(spans)
    qstar = max(qs, key=lambda q: spans[q][1] - spans[q][0])
    cmin = min(a for a, _ in spans.values())
    cmax = max(b for _, b in spans.values())
    rest = [q for q in qs if q != qstar]
    out = [(qstar, cmin, cmax, True, not rest)]
    for i, q in enumerate(rest):
        a, b = spans[q]
        out.append((q, a, b, False, i == len(rest) - 1))
    return out


def _schedule(plans):
    """Emission-ordered list of (chain, q, w0, c0, c1, start, stop,
    chain_done) where chain = (s, I, cl, ch).

    Chains in a group are interleaved round-robin by window index, which
    preserves each chain's start-first order while pacing x-chunk arrival.
    """
    sched = []
    for gi, chains in enumerate(GROUPS):
        wins = {c: _chain_windows(plans[c[0]], c[1], c[2], c[3]) for c in chains}
        if gi <= 1:
            # the opening groups run while x pieces are still arriving;
            # serial chains consume chunks in arrival order (round-robin
            # would front-load all four interior start-chunks)
            order = [(c, i) for c in chains for i in range(len(wins[c]))]
        else:
            maxlen = max(len(v) for v in wins.values())
            order = [
                (c, step)
                for step in range(maxlen)
                for c in chains
                if step < len(wins[c])
            ]
        for c, step in order:
            q, c0, c1, st, sp = wins[c][step]
            s, I = c[0], c[1]
            w0 = plans[s]["C"] + plans[s]["off"] - (128 * q - 512 * I)
            sched.append((c, q, w0, c0, c1, st, sp, step == len(wins[c]) - 1))
    return sched


_CONST_CACHE = None


def _consts():
    global _CONST_CACHE
    if _CONST_CACHE is None:
        import ml_dtypes

        plans = _plan()
        wbuf = np.concatenate(
            [_toeplitz(p["G"], p["C"], p["W"]) for p in plans], axis=1
        ).astype(ml_dtypes.bfloat16)
        _CONST_CACHE = (plans, np.ascontiguousarray(wbuf))
    return _CONST_CACHE


# ----------------------------------------------------------------- program
_NC_CACHE = None


def _w_pieces(plans, sched):
    """Split the concatenated W buffer into DMA pieces, ordered by first use.

    Scale 0's first piece is cut to exactly cover its first two schedule
    steps so the PE can start ~1us sooner; other scales use 2 halves
    (few pieces -> few DMA sems -> short semaphore-teardown epilogue).
    """
    bases = []
    b = 0
    for p in plans:
        bases.append(b)
        b += p["W"]
    pieces = []  # (first_use_idx, lo, hi) in concat cols
    for s, p in enumerate(plans):
        # only the columns the windowed schedule actually reads
        rd = [(w0 + c0, w0 + c1) for c, q, w0, c0, c1, st, sp, cd in sched if c[0] == s]
        rlo = min(a for a, b in rd)
        rhi = max(b for a, b in rd)
        npieces = max(1, round((rhi - rlo) / 1024))
        step = (((rhi - rlo) // npieces) + 127) & ~127
        cuts = list(range(rlo, rhi, step)) + [rhi]
        if s == 0:
            # extra cut so the opening half-chain's first read is its own
            # tiny piece
            f = sched[0]
            assert f[0][0] == 0
            fl, fh = f[2] + f[3], f[2] + f[4]
            cuts = sorted({rlo, fl, fh, rhi} | set(cuts))
        for lo, hi in zip(cuts[:-1], cuts[1:]):
            first = None
            for i, (c, q, w0, c0, c1, st, sp, cd) in enumerate(sched):
                if c[0] == s and w0 + c0 < hi and w0 + c1 > lo:
                    first = i
                    break
            pieces.append((first if first is not None else len(sched), bases[s] + lo, bases[s] + hi))
    pieces.sort()
    return bases, [(lo, hi) for _, lo, hi in pieces]


def _build_program():
    import concourse.bass as bass
    import concourse.bacc as bacc
    import concourse.mybir as mybir
    import concourse.tile as tile

    plans, _ = _consts()
    sched = _schedule(plans)
    wtot = sum(p["W"] for p in plans)
    bases, wpieces = _w_pieces(plans, sched)

    nc = bacc.Bacc(None, target_bir_lowering=False, debug=False)

    x_d = nc.declare_dram_parameter("x", [128, NQ * BPC], mybir.dt.bfloat16, isOutput=False)
    w_d = nc.declare_dram_parameter("w", [128, wtot], mybir.dt.bfloat16, isOutput=False)
    # outputs leave as bf16 (host upcasts): halves the store bytes; adds
    # ~0.1% rms quantization vs the 2e-2 budget
    out_d = nc.declare_dram_parameter(
        "out", [NSC, 128, T], mybir.dt.bfloat16, isOutput=True
    )

    # ring for each whole-scale output store (sync carries x early, scalar
    # carries W early; both are free by the time these flow mid-schedule)
    OUT_ENG = {1: "sync", 3: "sync", 4: "scalar", 2: "scalar"}
    # (0,2) has the most windows in the final group, so round-robin emission
    # makes it the very last chain to finish
    LAST_CHAIN = (0, 2, 0, 512)

    with tile.TileContext(nc) as tc:
        with (
            tc.tile_pool(name="xp", bufs=1) as xp,
            tc.tile_pool(name="wp", bufs=1) as wp,
            tc.tile_pool(name="op", bufs=1) as op,
            tc.tile_pool(name="pp", bufs=1, space=bass.MemorySpace.PSUM) as pp,
        ):
            # x chunks on the sync (SP) HWDGE ring in consumption order
            xsb = xp.tile([128, NQ * BPC], mybir.dt.bfloat16, tag="xsb", name="xsb")
            for g0, g1 in ((0, 3), (3, 7), (7, 11), (11, NQ)):
                nc.sync.dma_start(
                    xsb[:, g0 * BPC : g1 * BPC],
                    x_d[:, g0 * BPC : g1 * BPC],
                )

            # W pieces in first-use order.  The stream is 2.3MB — one ring
            # can't keep ahead of the warmed-up PE — so after the first
            # three critical pieces (scalar ring, whose queue is empty),
            # alternate pieces onto the sync ring behind the short x stream.
            wsb = wp.tile([128, wtot], mybir.dt.bfloat16, tag="wsb", name="wsb")
            for i, (lo, hi) in enumerate(wpieces):
                eng = nc.scalar if (i < 3 or i % 2 == 1) else nc.sync
                eng.dma_start(wsb[:, lo:hi], w_d[:, lo:hi])

            # PE clock warmup: the PE sits idle ~2.5us waiting for the first
            # DMA while DVFS has it at low clock (first real matmuls
            # otherwise run 370-700ns instead of 216).  Dummy matmuls on a
            # zeroed scratch tile ramp it; every real chain opens with
            # start=True so the scratch psum bank contents never leak.
            warm = xp.tile([128, 512], mybir.dt.bfloat16, tag="warm", name="warm")
            nc.gpsimd.memset(warm[:], 0.0)
            wps = pp.tile([128, 512], mybir.dt.float32, tag="ps7", name="ps_warm")
            for _ in range(4):
                nc.tensor.matmul(
                    wps[:], warm[:, 0:128], warm[:], start=True, stop=True
                )

            stgs = [
                op.tile([128, T], mybir.dt.bfloat16, tag=f"stg{s}", name=f"stg{s}")
                for s in range(NSC)
            ]

            psums = {}
            for gi, chains in enumerate(GROUPS):
                for ci, c in enumerate(chains):
                    psums[c] = pp.tile(
                        [128, 512],
                        mybir.dt.float32,
                        tag=f"ps{GROUP_TAGS[gi][ci]}",
                        name=f"ps_{c[0]}_{c[1]}_{c[2]}",
                    )

            done = {s: 0 for s in range(NSC)}
            for c, q, w0, c0, c1, start, stop, chain_done in sched:
                s, I, cl, ch = c
                nc.tensor.matmul(
                    psums[c][:, c0:c1],
                    xsb[:, q * BPC : (q + 1) * BPC],
                    wsb[:, bases[s] + w0 + c0 : bases[s] + w0 + c1],
                    start=start,
                    stop=stop,
                )
                if not chain_done:
                    continue
                stg = stgs[s]
                t0c = 512 * I
                done[s] += 1
                if c == LAST_CHAIN:
                    # final chain: copy halves, stores on both rings, to
                    # shorten the tail
                    nc.vector.tensor_copy(
                        stg[:, t0c : t0c + 256], psums[c][:, 0:256]
                    )
                    nc.vector.tensor_copy(
                        stg[:, t0c + 256 : t0c + 512], psums[c][:, 256:512]
                    )
                    nc.sync.dma_start(
                        out_d[s][:, t0c : t0c + 256], stg[:, t0c : t0c + 256]
                    )
                    nc.scalar.dma_start(
                        out_d[s][:, t0c + 256 : t0c + 512],
                        stg[:, t0c + 256 : t0c + 512],
                    )
                    continue
                nc.vector.tensor_copy(
                    stg[:, t0c + cl : t0c + ch], psums[c][:, cl:ch]
                )
                if s == 0:
                    # scale 1 brackets the schedule; store per-piece
                    nc.sync.dma_start(
                        out_d[s][:, t0c + cl : t0c + ch], stg[:, t0c + cl : t0c + ch]
                    )
                elif done[s] == NI:
                    # whole-scale store once the last block is staged
                    eng = nc.sync if OUT_ENG[s] == "sync" else nc.scalar
                    eng.dma_start(out_d[s], stg[:])

    nc.compile()
    return nc


def _program():
    global _NC_CACHE
    if _NC_CACHE is None:
        _NC_CACHE = _build_program()
    return _NC_CACHE


# ----------------------------------------------------------------- entry
def kernel(x: np.ndarray) -> np.ndarray:
    """x: [16, 2048, 64] float32 -> [16, 2048, 64, 5] float32"""
    global LAST_EXEC_NS
    import ml_dtypes
    from concourse.bass_utils import run_bass_kernel_spmd

    x = np.asarray(x)
    n, t, c = x.shape
    assert (t, n * c) == (T, B), (x.shape,)

    X = x.transpose(1, 0, 2).reshape(T, B).astype(np.float32)
    _, wbuf = _consts()
    in_maps = []
    for core in range(N_CORES):
        xc = X[:, core * BPC : (core + 1) * BPC]  # [2048, 128]
        xc = (
            xc.reshape(NQ, 128, BPC)
            .transpose(1, 0, 2)
            .reshape(128, NQ * BPC)
            .astype(ml_dtypes.bfloat16)
        )
        in_maps.append({"x": np.ascontiguousarray(xc), "w": wbuf})

    nc = _program()
    trace = bool(int(os.environ.get("CWT_TRACE", "0")))
    res = run_bass_kernel_spmd(nc, in_maps, list(range(N_CORES)), trace=trace)
    if trace:
        LAST_EXEC_NS = res.exec_time_ns
        globals()["LAST_RESULTS"] = res

    # per-core out: [5, 128, 2048] bf16 (b-local, t) -> Y [5, T, B] fp32
    Y = np.empty((NSC, T, B), np.float32)
    for core in range(N_CORES):
        o = np.asarray(res.results[core]["out"]).astype(np.float32)
        Y[:, :, core * BPC : (core + 1) * BPC] = o.transpose(0, 2, 1)
    return np.ascontiguousarray(
        Y.reshape(NSC, T, n, c).transpose(2, 1, 3, 0).astype(np.float32)
    )


# revision 40
# speedup vs baseline: 1.1949x; 1.1775x over previous
"""CWT (continuous wavelet transform, pywt 'morl', 5 scales) as a Bass/Tile
kernel for 8 Trainium2 NeuronCores.

Math: for each scale s with integrated-wavelet filter k (length L), the
reference computes  trim(diff(full_corr(x, k))) * (-sqrt(s)) along T.  That
whole pipeline is a single correlation with the fixed kernel
    G[j] = sqrt(s) * (k[j] - k[j-1]),  j = 0..L  (k[-1] = k[L] = 0)
applied with offset  off = floor((L-2)/2) - (L-1):
    y[t] = sum_j x[t + off + j] * G[j]   (x zero-padded outside [0,T))
i.e. y = A_s @ x with the Toeplitz band matrix A_s[t, u] = G[u - t - off].

Kernel strategy v2 (SPMD over 8 cores): pure B-sharding.  Core c owns the
128 batch*channel columns [128c, 128c+128); every core computes all 2048
t_out rows for its columns.  All t_out-block indices are then
core-independent, so a single static instruction stream works with NO
per-core shifted data: the banded scales read x chunks directly (chunks
outside [0,16) are the zero-padding and are simply dropped).

All matmul operands are bfloat16 (full PE rate, half the DMA bytes of
fp32r); outputs leave as bf16 too (host upcasts).  ~2.9e-3 rel err vs the
2e-2 budget.  Per (scale, t_out block I):
    psum[b, c0:c1] += X_chunk[q].T @ Wsc[:, w0(s,I,q)+c0 : w0+c1]
accumulated over the chunks q that intersect the band, where Wsc is a
per-scale Toeplitz sliding window identical on every core, and [c0, c1)
is the chunk's exact nonzero band span (PE cost is streamed columns, so
skipping the zero wings of the narrow-band scales cuts the stream from
113k to 92.5k columns; 220 matmuls/core).  start_tensor_calc resets the
whole PSUM bank, so each chain starts with its widest-window chunk
(accumulation commutes) widened to the chain's full column union.

DMA: x (0.5MB) + W (2.3MB, only the band columns actually read) in bf16,
consumption-ordered pieces split across the two HWDGE rings (sync=x+most
outs, scalar=W+late outs); outputs are staged psum->SBUF (vector cast to
bf16) and stored as soon as each chain stops — per-block pieces for
scale 1 (which brackets the schedule), whole-scale stores otherwise.
The dense scales run mid-schedule so their output bursts drain while
later groups compute; the final chain's copy+store is split across both
rings to minimize the tail.
"""
import sys
import os

sys.path.insert(0, "/opt/trn_rl_repo")

import numpy as np

# ----------------------------------------------------------------- constants
WIDTHS = [1, 27, 76, 167, 336]
T = 2048
B = 1024  # 16 batch * 64 channels
N_CORES = 8
BPC = B // N_CORES  # 128 batch*channel columns per core
NQ = T // 128  # 16 t_in chunks
NI = T // 512  # 4 t_out blocks per core (all computed by every core)
NSC = len(WIDTHS)

LAST_EXEC_NS = None  # set when CWT_TRACE=1


def _filters():
    """pywt 'morl' integrated wavelet, resampled per scale (matches reference)."""
    precision = 10
    n = 2**precision
    lb, ub = -8.0, 8.0
    t = np.linspace(lb, ub, n)
    psi = np.exp(-(t**2) / 2.0) * np.cos(5.0 * t)
    step = t[1] - t[0]
    int_psi = np.cumsum(psi) * step
    filts = []
    for scale in WIDTHS:
        j = (np.arange(scale * (ub - lb) + 1) / (scale * step)).astype(np.int64)
        j = j[j < n]
        filts.append(int_psi[j].astype(np.float32))
    return filts


def _g_kernels():
    """Effective correlation kernels G_s (len L+1) and offsets off_s."""
    gs = []
    for s, k in zip(WIDTHS, _filters()):
        k64 = k.astype(np.float64)
        L = len(k64)
        G = (np.sqrt(s) * np.diff(np.concatenate([[0.0], k64, [0.0]]))).astype(
            np.float32
        )
        off = int(np.floor((L - 2) / 2.0)) - (L - 1)
        gs.append((G, off))
    return gs


def _plan():
    """Per-scale Toeplitz window geometry + per-block chunk ranges.

    w0(s, I, q) = C_s + off_s - (128q - 512I) is the first W column of the
    512-wide rhs slice for chunk q of t_out block I.
    """
    plans = []
    for G, off in _g_kernels():
        L1 = len(G)
        qr = []
        vs = []
        for I in range(NI):
            lo = max(0, (512 * I + off) // 128)
            hi = min(NQ - 1, (512 * I + 511 + off + L1 - 1) // 128)
            qr.append((lo, hi))
            vs += [128 * q - 512 * I for q in range(lo, hi + 1)]
        C = max(vs) - off
        W = max(vs) - min(vs) + 512
        plans.append({"off": off, "L1": L1, "qr": qr, "C": C, "W": W, "G": G})
    return plans


def _toeplitz(G, C, W):
    p = np.arange(128)[:, None]
    w = np.arange(W)[None, :]
    idx = p - w + C
    valid = (idx >= 0) & (idx < len(G))
    return np.where(valid, G[np.clip(idx, 0, len(G) - 1)], np.float32(0.0)).astype(
        np.float32
    )


# chain processing groups: (scale, t_out block).  Scale 1's short chains
# bracket the schedule: two start it (smallest W piece -> earliest first
# matmul) and two end it (shortest chains -> smallest output tail).  The
# dense scales run mid-schedule so their 4-chains-end-together output
# bursts drain while later groups compute.
# chain spec: (scale, t_out block, psum col lo, col hi).  The opening
# chain (0,0) is split into two half-width chains so the first W DMA piece
# on the critical path to the first matmul is only 256 cols.
GROUPS = [
    [(0, 0, 0, 256), (0, 0, 256, 512), (0, 1, 0, 512)],
    [(1, 0, 0, 512), (1, 1, 0, 512), (1, 2, 0, 512), (1, 3, 0, 512)],
    [(3, 0, 0, 512), (3, 1, 0, 512), (3, 2, 0, 512), (3, 3, 0, 512)],
    [(4, 0, 0, 512), (4, 1, 0, 512), (4, 2, 0, 512), (4, 3, 0, 512)],
    [(2, 0, 0, 512), (2, 1, 0, 512), (2, 2, 0, 512), (2, 3, 0, 512)],
    [(0, 2, 0, 512), (0, 3, 0, 512)],
]
# psum bank tag per chain, alternating the two 4-bank halves between
# consecutive groups
GROUP_TAGS = [[0, 1, 2], [4, 5, 6, 7], [0, 1, 2, 3], [4, 5, 6, 7], [0, 1, 2, 3], [4, 5]]


def _chain_windows(p, I, cl=0, ch=512):
    """Per-chunk banded column windows for one (scale, block) chain,
    restricted to psum cols [cl, ch).

    Returns [(q, c0, c1, start, stop)]: matmul psum cols [c0, c1), skipping
    the all-zero columns of the Toeplitz band slice (big win for the
    narrow-band scales).
    """
    C, L1, off = p["C"], p["L1"], p["off"]
    nz_lo, nz_hi = C - L1 + 1, C + 128  # nonzero W cols [lo, hi)
    lo, hi = p["qr"][I]
    spans = {}
    for q in range(lo, hi + 1):
        w0 = C + off - (128 * q - 512 * I)
        a = max(cl, nz_lo - w0)
        b = min(ch, nz_hi - w0)
        if b <= a:
            continue
        spans[q] = (a, b)  # exact band span; matmul N needs no alignment
    # start_tensor_calc resets the ENTIRE psum bank on hardware, so each
    # chain gets exactly one start.  Accumulation is commutative: lead with
    # the chunk whose band window is widest (interior chunks cover the full
    # 512, so widening the start to the chain's column union is ~free) and
    # append the remaining chunks in q order.  stop rides the final matmul.
    qs = list(spans)
    qstar = max(qs, key=lambda q: spans[q][1] - spans[q][0])
    cmin = min(a for a, _ in spans.values())
    cmax = max(b for _, b in spans.values())
    rest = [q for q in qs if q != qstar]
    out = [(qstar, cmin, cmax, True, not rest)]
    for i, q in enumerate(rest):
        a, b = spans[q]
        out.append((q, a, b, False, i == len(rest) - 1))
    return out


def _schedule(plans):
    """Emission-ordered list of (chain, q, w0, c0, c1, start, stop,
    chain_done) where chain = (s, I, cl, ch).

    Chains in a group are interleaved round-robin by window index, which
    preserves each chain's start-first order while pacing x-chunk arrival.
    """
    sched = []
    for gi, chains in enumerate(GROUPS):
        wins = {c: _chain_windows(plans[c[0]], c[1], c[2], c[3]) for c in chains}
        if gi <= 1:
            # the opening groups run while x pieces are still arriving;
            # serial chains consume chunks in arrival order (round-robin
            # would front-load all four interior start-chunks)
            order = [(c, i) for c in chains for i in range(len(wins[c]))]
        else:
            maxlen = max(len(v) for v in wins.values())
            order = [
                (c, step)
                for step in range(maxlen)
                for c in chains
                if step < len(wins[c])
            ]
        for c, step in order:
            q, c0, c1, st, sp = wins[c][step]
            s, I = c[0], c[1]
            w0 = plans[s]["C"] + plans[s]["off"] - (128 * q - 512 * I)
            sched.append((c, q, w0, c0, c1, st, sp, step == len(wins[c]) - 1))
    return sched


_CONST_CACHE = None


def _consts():
    global _CONST_CACHE
    if _CONST_CACHE is None:
        import ml_dtypes

        plans = _plan()
        wbuf = np.concatenate(
            [_toeplitz(p["G"], p["C"], p["W"]) for p in plans], axis=1
        ).astype(ml_dtypes.bfloat16)
        _CONST_CACHE = (plans, np.ascontiguousarray(wbuf))
    return _CONST_CACHE


# ----------------------------------------------------------------- program
_NC_CACHE = None


def _w_pieces(plans, sched):
    """Split the concatenated W buffer into DMA pieces, ordered by first use.

    Scale 0's first piece is cut to exactly cover its first two schedule
    steps so the PE can start ~1us sooner; other scales use 2 halves
    (few pieces -> few DMA sems -> short semaphore-teardown epilogue).
    """
    bases = []
    b = 0
    for p in plans:
        bases.append(b)
        b += p["W"]
    pieces = []  # (first_use_idx, lo, hi) in concat cols
    for s, p in enumerate(plans):
        # only the columns the windowed schedule actually reads
        rd = [(w0 + c0, w0 + c1) for c, q, w0, c0, c1, st, sp, cd in sched if c[0] == s]
        rlo = min(a for a, b in rd)
        rhi = max(b for a, b in rd)
        npieces = max(1, round((rhi - rlo) / 1024))
        step = (((rhi - rlo) // npieces) + 127) & ~127
        cuts = list(range(rlo, rhi, step)) + [rhi]
        if s == 0:
            # extra cut so the opening half-chain's first read is its own
            # tiny piece
            f = sched[0]
            assert f[0][0] == 0
            fl, fh = f[2] + f[3], f[2] + f[4]
            cuts = sorted({rlo, fl, fh, rhi} | set(cuts))
        for lo, hi in zip(cuts[:-1], cuts[1:]):
            first = None
            for i, (c, q, w0, c0, c1, st, sp, cd) in enumerate(sched):
                if c[0] == s and w0 + c0 < hi and w0 + c1 > lo:
                    first = i
                    break
            pieces.append((first if first is not None else len(sched), bases[s] + lo, bases[s] + hi))
    pieces.sort()
    return bases, [(lo, hi) for _, lo, hi in pieces]


def _build_program():
    import concourse.bass as bass
    import concourse.bacc as bacc
    import concourse.mybir as mybir
    import concourse.tile as tile

    plans, _ = _consts()
    sched = _schedule(plans)
    wtot = sum(p["W"] for p in plans)
    bases, wpieces = _w_pieces(plans, sched)

    nc = bacc.Bacc(None, target_bir_lowering=False, debug=False)

    x_d = nc.declare_dram_parameter("x", [128, NQ * BPC], mybir.dt.bfloat16, isOutput=False)
    w_d = nc.declare_dram_parameter("w", [128, wtot], mybir.dt.bfloat16, isOutput=False)
    # outputs leave as bf16 (host upcasts): halves the store bytes; adds
    # ~0.1% rms quantization vs the 2e-2 budget
    out_d = nc.declare_dram_parameter(
        "out", [NSC, 128, T], mybir.dt.bfloat16, isOutput=True
    )

    # ring for each whole-scale output store (sync carries x early, scalar
    # carries W early; both are free by the time these flow mid-schedule)
    OUT_ENG = {1: "sync", 3: "sync", 4: "scalar", 2: "scalar"}
    # (0,2) has the most windows in the final group, so round-robin emission
    # makes it the very last chain to finish
    LAST_CHAIN = (0, 2, 0, 512)

    with tile.TileContext(nc) as tc:
        with (
            tc.tile_pool(name="xp", bufs=1) as xp,
            tc.tile_pool(name="wp", bufs=1) as wp,
            tc.tile_pool(name="op", bufs=1) as op,
            tc.tile_pool(name="pp", bufs=1, space=bass.MemorySpace.PSUM) as pp,
        ):
            # x chunks on the sync (SP) HWDGE ring in consumption order; the
            # last two pieces interleave with sync-carried W pieces below
            # (late x chunks aren't needed until mid-schedule)
            xsb = xp.tile([128, NQ * BPC], mybir.dt.bfloat16, tag="xsb", name="xsb")
            for g0, g1 in ((0, 3), (3, 7)):
                nc.sync.dma_start(
                    xsb[:, g0 * BPC : g1 * BPC],
                    x_d[:, g0 * BPC : g1 * BPC],
                )

            # W pieces in first-use order.  The stream is 2.3MB — one ring
            # can't keep ahead of the warmed-up PE — so after the first
            # three critical pieces (scalar ring, whose queue is empty),
            # alternate pieces onto the sync ring behind the short x stream.
            wsb = wp.tile([128, wtot], mybir.dt.bfloat16, tag="wsb", name="wsb")
            nsync = 0
            for i, (lo, hi) in enumerate(wpieces):
                eng = nc.scalar if (i < 3 or i % 2 == 1) else nc.sync
                eng.dma_start(wsb[:, lo:hi], w_d[:, lo:hi])
                if eng is nc.sync:
                    nsync += 1
                    if nsync == 1:
                        nc.sync.dma_start(
                            xsb[:, 7 * BPC : 11 * BPC], x_d[:, 7 * BPC : 11 * BPC]
                        )
                    elif nsync == 2:
                        nc.sync.dma_start(
                            xsb[:, 11 * BPC : NQ * BPC], x_d[:, 11 * BPC : NQ * BPC]
                        )

            # PE clock warmup: the PE sits idle ~2.5us waiting for the first
            # DMA while DVFS has it at low clock (first real matmuls
            # otherwise run 370-700ns instead of 216).  Dummy matmuls on a
            # zeroed scratch tile ramp it; every real chain opens with
            # start=True so the scratch psum bank contents never leak.
            warm = xp.tile([128, 512], mybir.dt.bfloat16, tag="warm", name="warm")
            nc.gpsimd.memset(warm[:], 0.0)
            wps = pp.tile([128, 512], mybir.dt.float32, tag="ps7", name="ps_warm")
            for _ in range(4):
                nc.tensor.matmul(
                    wps[:], warm[:, 0:128], warm[:], start=True, stop=True
                )

            stgs = [
                op.tile([128, T], mybir.dt.bfloat16, tag=f"stg{s}", name=f"stg{s}")
                for s in range(NSC)
            ]

            psums = {}
            for gi, chains in enumerate(GROUPS):
                for ci, c in enumerate(chains):
                    psums[c] = pp.tile(
                        [128, 512],
                        mybir.dt.float32,
                        tag=f"ps{GROUP_TAGS[gi][ci]}",
                        name=f"ps_{c[0]}_{c[1]}_{c[2]}",
                    )

            done = {s: 0 for s in range(NSC)}
            for c, q, w0, c0, c1, start, stop, chain_done in sched:
                s, I, cl, ch = c
                nc.tensor.matmul(
                    psums[c][:, c0:c1],
                    xsb[:, q * BPC : (q + 1) * BPC],
                    wsb[:, bases[s] + w0 + c0 : bases[s] + w0 + c1],
                    start=start,
                    stop=stop,
                )
                if not chain_done:
                    continue
                stg = stgs[s]
                t0c = 512 * I
                done[s] += 1
                if c == LAST_CHAIN:
                    # final chain: copy halves, stores on both rings, to
                    # shorten the tail
                    nc.vector.tensor_copy(
                        stg[:, t0c : t0c + 256], psums[c][:, 0:256]
                    )
                    nc.vector.tensor_copy(
                        stg[:, t0c + 256 : t0c + 512], psums[c][:, 256:512]
                    )
                    nc.sync.dma_start(
                        out_d[s][:, t0c : t0c + 256], stg[:, t0c : t0c + 256]
                    )
                    nc.scalar.dma_start(
                        out_d[s][:, t0c + 256 : t0c + 512],
                        stg[:, t0c + 256 : t0c + 512],
                    )
                    continue
                nc.vector.tensor_copy(
                    stg[:, t0c + cl : t0c + ch], psums[c][:, cl:ch]
                )
                if s == 0:
                    # scale 1 brackets the schedule; store per-piece
                    nc.sync.dma_start(
                        out_d[s][:, t0c + cl : t0c + ch], stg[:, t0c + cl : t0c + ch]
                    )
                elif done[s] == NI:
                    # whole-scale store once the last block is staged
                    eng = nc.sync if OUT_ENG[s] == "sync" else nc.scalar
                    eng.dma_start(out_d[s], stg[:])

    nc.compile()
    return nc


def _program():
    global _NC_CACHE
    if _NC_CACHE is None:
        _NC_CACHE = _build_program()
    return _NC_CACHE


# ----------------------------------------------------------------- entry
def kernel(x: np.ndarray) -> np.ndarray:
    """x: [16, 2048, 64] float32 -> [16, 2048, 64, 5] float32"""
    global LAST_EXEC_NS
    import ml_dtypes
    from concourse.bass_utils import run_bass_kernel_spmd

    x = np.asarray(x)
    n, t, c = x.shape
    assert (t, n * c) == (T, B), (x.shape,)

    X = x.transpose(1, 0, 2).reshape(T, B).astype(np.float32)
    _, wbuf = _consts()
    in_maps = []
    for core in range(N_CORES):
        xc = X[:, core * BPC : (core + 1) * BPC]  # [2048, 128]
        xc = (
            xc.reshape(NQ, 128, BPC)
            .transpose(1, 0, 2)
            .reshape(128, NQ * BPC)
            .astype(ml_dtypes.bfloat16)
        )
        in_maps.append({"x": np.ascontiguousarray(xc), "w": wbuf})

    nc = _program()
    trace = bool(int(os.environ.get("CWT_TRACE", "0")))
    res = run_bass_kernel_spmd(nc, in_maps, list(range(N_CORES)), trace=trace)
    if trace:
        LAST_EXEC_NS = res.exec_time_ns
        globals()["LAST_RESULTS"] = res

    # per-core out: [5, 128, 2048] bf16 (b-local, t) -> Y [5, T, B] fp32
    Y = np.empty((NSC, T, B), np.float32)
    for core in range(N_CORES):
        o = np.asarray(res.results[core]["out"]).astype(np.float32)
        Y[:, :, core * BPC : (core + 1) * BPC] = o.transpose(0, 2, 1)
    return np.ascontiguousarray(
        Y.reshape(NSC, T, n, c).transpose(2, 1, 3, 0).astype(np.float32)
    )
